# revision 27
# baseline (speedup 1.0000x reference)
"""Trainium2 Bass kernel for nn_Attention_49366354100559.

Multi-head attention: B=2, T=2048, D=768, H=12, Dh=64. The reference
zeroes the upper triangle of scores (not -inf) before softmax, so masked
positions contribute exp(0)=1 to both numerator and denominator.

Sharding: 8 cores = 2 batches x 4 core-groups; each core computes 3
heads of one batch and produces a partial [2048, 768] output
(pre-W_o-bias, x32 scaled); host sums the partials, rescales, adds b_o.

Precision design (rel-err budget ~7e-3 vs 2e-2 gate): per-element
relative noise propagates unchanged through random-sign contractions,
so every tensor on the V->output path is fp16 (e, vaug, suffix sums,
aout, W_o) and the V projection uses a 3-term fp8 residual expansion
(x1@W1 + x2@W1 + x1@W2) to cancel x/W quantization noise. The Q/K path
is damped by the 1/8 temperature and softmax, so x, W_q/k and q/k stay
fp8, which keeps those projections on DoubleRow matmuls (2 contraction
tiles folded per pass at 0.5 cycles/column).

Device structure:
  - x arrives host-transposed/cast to fp8 in two pair-interleaved
    layouts (QK moving operand; V stationary operand) -- no on-device
    transposes. The x quantization residual ships alongside for the V
    projection.
  - Scores^T[k, q] per 128-k-tile: plain fp8 matmuls into [128, 1024]
    psum chunks; exp on ACT straight out of PSUM (fused 1/(8*32*32)
    scale) into fp16 e-tiles; causal edge fixed by affine_select
    (fill=1) on gpsimd. Chunks are 512-aligned; garbage columns below
    the band are exp'd then overwritten by the select.
  - attn@v: plain fp16 matmuls, vaug carries a 65th all-ones column so
    the softmax denominator accumulates in the same psum rows. Fully
    masked k-quads fold in via one fp16 suffix-sum matmul (the suffix
    tile's 65th column carries the masked count).
  - finalize: fast reciprocal of row 64, partition-broadcast,
    multiply -> aout fp16 (h0/h1 stacked into one 128-row tile via a
    partition-shift DMA, h2 separate 64-row tile).
  - O-projection: fp16, two stationaries (aout01, aout2) per token
    tile; psum -> fp16 staging -> DMA out.
"""

import sys

import numpy as np

if "/opt/trn_rl_repo" not in sys.path:
    sys.path.insert(0, "/opt/trn_rl_repo")

import ml_dtypes

import concourse.mybir as mybir
from concourse import bacc
from concourse.tile import TileContext
from concourse.bass_utils import run_bass_kernel_spmd

F32 = mybir.dt.float32
F16 = mybir.dt.float16
F8 = mybir.dt.float8e4
AF = mybir.ActivationFunctionType
ALU = mybir.AluOpType
DR = mybir.MatmulPerfMode.DoubleRow
NP8 = ml_dtypes.float8_e4m3

N_CORES = 8
T = 2048
D = 768
HPC = 3   # heads per core
DH = 64
NK = 16   # k-token tiles of 128
NG = 4    # q/token groups of 512
WS = 32.0          # weight scale (W' = 32 W)
ESCALE = 0.125 / (WS * WS)


def build_nc():
    nc = bacc.Bacc("TRN2", target_bir_lowering=False, debug=False,
                   num_devices=N_CORES)
    d = {}
    ei = "ExternalInput"
    d["xq"] = nc.dram_tensor("xq", [128, 12288], F8, kind=ei).ap()
    d["xv"] = nc.dram_tensor("xv", [128, 12288], F8, kind=ei).ap()
    d["wqk"] = nc.dram_tensor("wqk", [128, 2304], F8, kind=ei).ap()
    d["wv"] = nc.dram_tensor("wv", [128, 1170], F8, kind=ei).ap()
    d["wv2"] = nc.dram_tensor("wv2", [128, 1170], F8, kind=ei).ap()
    d["woa"] = nc.dram_tensor("woa", [128, 768], F16, kind=ei).ap()
    d["wob"] = nc.dram_tensor("wob", [64, 768], F16, kind=ei).ap()
    d["bqk"] = nc.dram_tensor("bqk", [128, 3], F32, kind=ei).ap()
    d["bv"] = nc.dram_tensor("bv", [128, 195], F32, kind=ei).ap()
    d["ones"] = nc.dram_tensor("ones", [128, 512], F16, kind=ei).ap()
    d["y"] = nc.dram_tensor("y", [T, D], F16, kind="ExternalOutput").ap()
    import os
    if os.environ.get("ATTN_DEBUG"):
        eo = "ExternalOutput"
        for g in range(3):
            d[f"dbg_qkt{g}"] = nc.dram_tensor(
                f"dbg_qkt{g}", [128, T], F8, kind=eo).ap()
        for h in range(3):
            d[f"dbg_vaug{h}"] = nc.dram_tensor(
                f"dbg_vaug{h}", [128, NK * 65], F16, kind=eo).ap()
        d["dbg_aout01"] = nc.dram_tensor(
            "dbg_aout01", [128, T], F16, kind=eo).ap()
        d["dbg_aout2"] = nc.dram_tensor(
            "dbg_aout2", [64, T], F16, kind=eo).ap()
        d["dbg_e"] = nc.dram_tensor(
            "dbg_e", [128, 16 * T], F16, kind=eo).ap()
        d["dbg_pnd"] = nc.dram_tensor(
            "dbg_pnd", [128, 4 * 512], F32, kind=eo).ap()
        d["dbg_rb"] = nc.dram_tensor(
            "dbg_rb", [64, 4 * 512], F32, kind=eo).ap()

    with TileContext(nc) as tc:
        _emit(nc, tc, d)
    nc.compile()
    return nc


def _emit(nc, tc, d):
    from contextlib import ExitStack

    with ExitStack() as ctx:
        wp = ctx.enter_context(tc.tile_pool(name="wp", bufs=1))
        main = ctx.enter_context(tc.tile_pool(name="main", bufs=1))

        # ---- weights / constants ----
        wqk = wp.tile([128, 2304], F8, tag="wqk", name="wqk")
        wv = wp.tile([128, 1170], F8, tag="wv", name="wv")
        wv2 = wp.tile([128, 1170], F8, tag="wv2", name="wv2")
        woa = wp.tile([128, 768], F16, tag="woa", name="woa")
        wob = wp.tile([64, 768], F16, tag="wob", name="wob")
        bqk = wp.tile([128, 3], F32, tag="bqk", name="bqk")
        bv = wp.tile([128, 195], F32, tag="bv", name="bv")
        ones = wp.tile([128, 512], F16, tag="ones", name="ones")

        # ---- persistent SBUF ----
        qkt = [main.tile([128, T], F8, tag=f"qkt{g}", name=f"qkt{g}")
               for g in range(3)]
        alt2 = main.tile([128, T], F8, tag="alt2", name="alt2")
        # vaug[h]: [128, 16, 65] k-tile-major v values + ones column
        vaug = [main.tile([128, NK * 65], F16, tag=f"vaug{h}",
                          name=f"vaug{h}") for h in range(HPC)]
        vsum = [[main.tile([128, 65], F16, tag=f"vs{h}{g}",
                           name=f"vs{h}{g}") for g in range(3)]
                for h in range(HPC)]
        aout01 = main.tile([128, T], F16, tag="aout01", name="aout01")
        aout2 = main.tile([64, T], F16, tag="aout2", name="aout2")
        a1tmp = main.tile([64, T], F16, tag="a1tmp", name="a1tmp")

        at_ctx = ExitStack()
        ep = at_ctx.enter_context(tc.tile_pool(name="ep", bufs=20))
        fin = at_ctx.enter_context(tc.tile_pool(name="fin", bufs=2))
        yst = at_ctx.enter_context(tc.tile_pool(name="yst", bufs=3))
        sps = at_ctx.enter_context(
            tc.tile_pool(name="sps", bufs=2, space="PSUM"))
        pools = {}

        # ============ phase 0: DMAs ============
        xp_ctx = ExitStack()
        xp = xp_ctx.enter_context(tc.tile_pool(name="xp", bufs=1))
        xq = xp.tile([128, 12288], F8, tag="xq", name="xq")
        xv = xp.tile([128, 12288], F8, tag="xv", name="xv")

        s4 = lambda p: slice(4096 * p, 4096 * (p + 1))
        nc.sync.dma_start(xq[:, s4(0)], d["xq"][:, s4(0)])
        nc.gpsimd.dma_start(xq[:, s4(1)], d["xq"][:, s4(1)])
        nc.scalar.dma_start(wqk[:], d["wqk"])
        nc.scalar.dma_start(xq[:, s4(2)], d["xq"][:, s4(2)])
        nc.scalar.dma_start(bqk[:], d["bqk"])
        nc.sync.dma_start(xv[:, s4(0)], d["xv"][:, s4(0)])
        nc.gpsimd.dma_start(xv[:, s4(1)], d["xv"][:, s4(1)])
        nc.sync.dma_start(xv[:, s4(2)], d["xv"][:, s4(2)])
        nc.scalar.dma_start(wv[:], d["wv"])
        nc.scalar.dma_start(wv2[:], d["wv2"])
        nc.scalar.dma_start(bv[:], d["bv"])
        nc.scalar.dma_start(ones[:], d["ones"])
        nc.scalar.dma_start(woa[:], d["woa"])
        nc.scalar.dma_start(wob[:], d["wob"])

        # PE p-state warmup: ~3us of dummy matmuls while input DMAs land
        warm = wp.tile([128, 512], F16, tag="warm", name="warm")
        nc.vector.memset(warm[:], 1.0)
        wps = sps.tile([128, 1024], F32, tag="s", name="warmps")
        for i in range(16):
            nc.tensor.matmul(wps[:, 0:256], warm[:, 0:128], warm[:, 0:256],
                             start=True, stop=True)

        def pair(t, s, w):
            return t[:, s:s + 2 * w].rearrange("p (two c) -> p two c", two=2)

        # ============ projections ============
        pj_ctx = ExitStack()
        pj = pj_ctx.enter_context(
            tc.tile_pool(name="pj", bufs=4, space="PSUM"))

        def qk_proj(n, g):
            ps = pj.tile([128, 512], F32, tag="ps", name=f"qk{g}_{n}")
            for p in range(3):
                nc.tensor.matmul(
                    ps[:], pair(wqk, (3 * p + g) * 256, 128),
                    pair(xq, (4 * p + n) * 1024, 512),
                    start=(p == 0), stop=(p == 2), perf_mode=DR)
            nc.vector.tensor_scalar_add(
                qkt[g][:, 512 * n:512 * (n + 1)], ps[:], bqk[:, g:g + 1])

        def v_proj(tt):
            ps = pj.tile([128, 195], F32, tag="ps", name=f"v{tt}",
                         padded_shape=[128, 512])
            for i, wt in enumerate((wv, wv2)):
                for p in range(3):
                    nc.tensor.matmul(
                        ps[:], pair(xv, (16 * p + tt) * 256, 128),
                        pair(wt, 390 * p, 195),
                        start=(i == 0 and p == 0),
                        stop=(i == 1 and p == 2), perf_mode=DR)
            for h in range(HPC):
                nc.vector.tensor_add(
                    vaug[h][:, 65 * tt:65 * tt + 65],
                    ps[:, 65 * h:65 * h + 65], bv[:, 65 * h:65 * h + 65])

        def post_proj():
            nc.sync.dma_start(alt2[0:64, :], qkt[2][64:128, :])
            nc.sync.dma_start(alt2[64:128, :], qkt[2][0:64, :])
            for h in range(HPC):
                va = vaug[h].rearrange("p (k c) -> p c k", c=65)
                with nc.allow_low_precision(
                        reason="suffix sum of <=12 fp16 values"):
                    for g in range(3):
                        nc.vector.tensor_reduce(
                            vsum[h][g][:], va[:, :, 4 * (g + 1):NK],
                            axis=mybir.AxisListType.X, op=ALU.add)

        def mkcl(fn, *a):
            return lambda: fn(*a)

        # ============ attention ============
        headqk = [
            (qkt[0][0:64, :], qkt[1][0:64, :]),
            (qkt[0][64:128, :], qkt[1][64:128, :]),
            (qkt[2][0:64, :], alt2[0:64, :]),
        ]
        fill1 = nc.gpsimd.to_reg(1.0)
        etiles = {}

        def scores_quad(h, J, pending=None, pmajor=False):
            qT, kT = headqk[h]
            chunks = []
            for j in range(4):
                lo = 128 * (4 * J + j)
                for P in range(lo // 1024, 2):
                    chunks.append((j, P))
            if pmajor:
                chunks.sort(key=lambda c: (c[1], c[0]))
            share = (((len(pending) + len(chunks) - 1) // len(chunks))
                     if pending else 0)
            for (j, P) in chunks:
                ki = 4 * J + j
                lo = 128 * ki
                if (h, ki) not in etiles or etiles[(h, ki)] is None:
                    etiles[(h, ki)] = ep.tile([128, T], F16, tag="e",
                                              name=f"e{h}_{ki}")
                e = etiles[(h, ki)]
                ps = sps.tile([128, 1024], F32, tag="s",
                              name=f"s{h}_{ki}_{P}")
                for nn in range(2):
                    s0 = 1024 * P + 512 * nn
                    if s0 + 512 <= lo:
                        continue
                    a0 = max(s0, lo)
                    nc.tensor.matmul(
                        ps[:, a0 - 1024 * P:512 * (nn + 1)],
                        kT[:, lo:lo + 128], qT[:, a0:s0 + 512])
                clo = max(lo, 1024 * P)
                nc.scalar.activation(
                    e[:, clo:1024 * (P + 1)],
                    ps[:, clo - 1024 * P:1024], AF.Exp, scale=ESCALE)
                if pending:
                    for _ in range(min(share, len(pending))):
                        pending.pop(0)()
                if P == 1:
                    w = 128 * (j + 1)
                    nc.gpsimd.affine_select(
                        e[:, 512 * J:512 * J + w],
                        e[:, 512 * J:512 * J + w],
                        pattern=[[1, w]], compare_op=ALU.is_ge,
                        fill=fill1, base=512 * J - lo,
                        channel_multiplier=-1)
            while pending:
                pending.pop(0)()

        def attnv_closures(h, g):
            pnd = pools["nd"].tile([128, 512], F32, tag="nd",
                                   name=f"nd{h}{g}")
            last = 4 * g + 3
            cl = []

            def mk(ki):
                def go():
                    nc.tensor.matmul(
                        pnd[0:65, :], vaug[h][:, 65 * ki:65 * ki + 65],
                        etiles[(h, ki)][:, 512 * g:512 * (g + 1)],
                        start=(ki == 0), stop=(ki == last and g == 3))
                return go

            for ki in range(4 * g + 4):
                cl.append(mk(ki))
            if g < 3:
                cl.append(lambda: nc.tensor.matmul(
                    pnd[0:65, :], vsum[h][g][:], ones[:],
                    start=False, stop=True))
            cl.append(lambda: finalize(h, g, pnd))
            return cl

        def attnv(h, g):
            for f in attnv_closures(h, g):
                f()

        def finalize(h, g, pnd):
            dnc = fin.tile([1, 512], F32, tag="dnc", name=f"dnc{h}{g}")
            rcp = fin.tile([1, 512], F32, tag="rcp", name=f"rcp{h}{g}")
            rb = fin.tile([64, 512], F32, tag="rb", name=f"rb{h}{g}")
            nc.vector.tensor_copy(dnc[:], pnd[64:65, :])
            nc.vector.reciprocal_approx_fast(rcp[:], dnc[:])
            nc.gpsimd.partition_broadcast(rb[:], rcp[:])
            import os
            if os.environ.get("ATTN_DEBUG") and h == 2:
                dbg = fin.tile([128, 512], F32, tag="dbgp",
                               name=f"dbgp{g}", bufs=4)
                nc.vector.tensor_copy(dbg[0:65, :], pnd[0:65, :])
                nc.sync.dma_start(d["dbg_pnd"][:, 512 * g:512 * (g + 1)],
                                  dbg[:])
                nc.sync.dma_start(d["dbg_rb"][:, 512 * g:512 * (g + 1)],
                                  rb[:])
            gs = slice(512 * g, 512 * (g + 1))
            if h == 0:
                nc.vector.tensor_mul(aout01[0:64, gs], pnd[0:64, :], rb[:])
            elif h == 1:
                nc.vector.tensor_mul(a1tmp[:, gs], pnd[0:64, :], rb[:])
                nc.sync.dma_start(aout01[64:128, gs], a1tmp[:, gs])
            else:
                nc.vector.tensor_mul(aout2[:, gs], pnd[0:64, :], rb[:])

        def oproj_closures(g):
            cl = []
            for tt in range(4 * g, 4 * g + 4):
                cl.append(lambda tt=tt: oproj_tt(tt))
            return cl

        def oproj(g):
            for f in oproj_closures(g):
                f()

        def oproj_tt(tt):
            if True:
                use_act = tt >= 12
                ts = slice(128 * tt, 128 * (tt + 1))
                yt = yst.tile([128, D], F16, tag="y", name=f"y{tt}")
                po = sps.tile([128, 1024], F32, tag="s", name=f"o{tt}")
                for (c0, w) in ((0, 512), (512, 256)):
                    nc.tensor.matmul(po[:, c0:c0 + w], aout01[:, ts],
                                     woa[:, c0:c0 + w],
                                     start=True, stop=False)
                    nc.tensor.matmul(po[:, c0:c0 + w], aout2[:, ts],
                                     wob[:, c0:c0 + w],
                                     start=False, stop=True)
                if use_act:
                    nc.scalar.activation(yt[:], po[:, 0:768], AF.Copy)
                else:
                    nc.vector.tensor_copy(yt[:], po[:, 0:768])
                nc.sync.dma_start(d["y"][ts, :], yt[:])

        # ---- emission schedule ----
        qk_proj(0, 0)
        qk_proj(0, 1)
        qk_proj(1, 0)
        qk_proj(1, 1)
        pend = [mkcl(qk_proj, 2, 0), mkcl(qk_proj, 2, 1),
                mkcl(qk_proj, 3, 0), mkcl(qk_proj, 3, 1)] + [
                mkcl(qk_proj, n, 2) for n in range(NG)]
        scores_quad(0, 0, pend, pmajor=True)
        scores_quad(0, 1, [mkcl(v_proj, tt) for tt in range(0, 8)])
        scores_quad(0, 2, [mkcl(v_proj, tt) for tt in range(8, 16)])
        post_proj()
        pj_ctx.close()
        xp_ctx.close()
        pools["nd"] = at_ctx.enter_context(
            tc.tile_pool(name="nd", bufs=4, space="PSUM"))

        scores_quad(0, 3)
        scores_quad(1, 0, attnv_closures(0, 0) + attnv_closures(0, 1))
        scores_quad(1, 1, attnv_closures(0, 2) + attnv_closures(0, 3))
        scores_quad(1, 2, attnv_closures(1, 0))
        scores_quad(1, 3, attnv_closures(1, 1))
        scores_quad(2, 0, attnv_closures(1, 2))
        scores_quad(2, 1, attnv_closures(1, 3))
        scores_quad(2, 2, attnv_closures(2, 0) + oproj_closures(0))
        scores_quad(2, 3, attnv_closures(2, 1) + attnv_closures(2, 2)
                    + oproj_closures(1) + oproj_closures(2))
        for f in attnv_closures(2, 3) + oproj_closures(3):
            f()
        import os
        if os.environ.get("ATTN_DEBUG"):
            for ki in range(16):
                nc.sync.dma_start(d["dbg_e"][:, T * ki:T * (ki + 1)],
                                  etiles[(2, ki)][:])
            for g in range(3):
                nc.sync.dma_start(d[f"dbg_qkt{g}"], qkt[g][:])
            for h2 in range(3):
                nc.sync.dma_start(d[f"dbg_vaug{h2}"], vaug[h2][:])
            nc.sync.dma_start(d["dbg_aout01"], aout01[:])
            nc.sync.dma_start(d["dbg_aout2"], aout2[:])
        at_ctx.close()


_NC_CACHE = None


def _get_nc():
    global _NC_CACHE
    if _NC_CACHE is None:
        _NC_CACHE = build_nc()
    return _NC_CACHE


def _f8(a):
    return np.clip(np.asarray(a, dtype=np.float32),
                   -240, 240).astype(NP8)


def _make_in_maps(residual_stream, W_q, b_q, W_k, b_k, W_v, b_v, W_o, b_o):
    in_maps = []
    for c in range(N_CORES):
        b = c // 4
        hs = [3 * (c % 4) + i for i in range(HPC)]
        cs = [slice(64 * h, 64 * h + 64) for h in hs]

        xT = np.ascontiguousarray(residual_stream[b].T)  # [768, 2048]
        xT8 = _f8(xT)
        # xq: [r, p*4096 + n*1024 + i*512 + t'] = xT[256p+128i+r, 512n+t']
        xq = (xT8.reshape(3, 2, 128, 4, 512).transpose(0, 2, 3, 1, 4)
              .reshape(3, 128, 4096).transpose(1, 0, 2).reshape(128, -1))

        # xv: [r, p*4096 + tt*256 + i*128 + t'] = xT[256p+128i+r, 128tt+t']
        def vlay(a):
            return (a.reshape(3, 2, 128, 16, 128).transpose(0, 2, 3, 1, 4)
                    .reshape(3, 128, 4096).transpose(1, 0, 2)
                    .reshape(128, -1))

        xv = vlay(xT8)

        Wcat = np.concatenate(
            [W_q[:, cs[0]], W_q[:, cs[1]], W_k[:, cs[0]], W_k[:, cs[1]],
             W_q[:, cs[2]], W_k[:, cs[2]]], axis=1) * WS  # [768, 384]
        wqk = _f8(Wcat.reshape(3, 2, 128, 3, 128)
                  .transpose(2, 0, 3, 1, 4).reshape(128, 2304))

        Vcat = np.zeros((D, 195), dtype=np.float32)
        for i, s in enumerate(cs):
            Vcat[:, 65 * i:65 * i + 64] = W_v[:, s] * WS
        wv8 = _f8(Vcat).astype(np.float32)
        wvr = _f8(Vcat - wv8).astype(np.float32)
        def wvlay(a):
            return np.ascontiguousarray(
                a.reshape(3, 2, 128, 195).transpose(2, 0, 1, 3)
                .reshape(128, 1170).astype(NP8))
        wvl = wvlay(wv8)
        wv2l = wvlay(wvr)

        bqk = np.stack(
            [np.concatenate([b_q[cs[0]], b_q[cs[1]]]),
             np.concatenate([b_k[cs[0]], b_k[cs[1]]]),
             np.concatenate([b_q[cs[2]], b_k[cs[2]]])],
            axis=1).astype(np.float32) * WS
        bvv = np.zeros((195,), dtype=np.float32)
        for i, s in enumerate(cs):
            bvv[65 * i:65 * i + 64] = b_v[s] * WS
            bvv[65 * i + 64] = 1.0
        bv = np.ascontiguousarray(
            np.broadcast_to(bvv[None, :], (128, 195)).astype(np.float32))

        Wo = np.concatenate([W_o[s, :] for s in cs], axis=0)  # [192, 768]
        woa = np.ascontiguousarray(Wo[0:128, :]).astype(np.float16)
        wob = np.ascontiguousarray(Wo[128:192, :]).astype(np.float16)

        in_maps.append({
            "xq": xq, "xv": xv, "wqk": wqk,
            "wv": wvl, "wv2": wv2l, "woa": woa, "wob": wob,
            "bqk": bqk, "bv": bv,
            "ones": np.ones((128, 512), dtype=np.float16),
        })
    return in_maps


def kernel(residual_stream, W_q, b_q, W_k, b_k, W_v, b_v, W_o, b_o,
           _trace=False):
    residual_stream = np.asarray(residual_stream, dtype=np.float32)
    args = [np.asarray(a, dtype=np.float32)
            for a in (W_q, b_q, W_k, b_k, W_v, b_v, W_o, b_o)]
    W_q, b_q, W_k, b_k, W_v, b_v, W_o, b_o = args
    nc = _get_nc()
    in_maps = _make_in_maps(residual_stream, W_q, b_q, W_k, b_k, W_v, b_v,
                            W_o, b_o)
    res = run_bass_kernel_spmd(nc, in_maps, core_ids=list(range(N_CORES)),
                               trace=_trace)
    B = residual_stream.shape[0]
    out = np.zeros((B, T, D), dtype=np.float32)
    for c in range(N_CORES):
        out[c // 4] += res.results[c]["y"].astype(np.float32)
    out /= WS
    out += b_o[None, None, :]
    if _trace:
        kernel._last_result = res
    return out


# revision 28
# speedup vs baseline: 1.0169x; 1.0169x over previous
"""Trainium2 Bass kernel for nn_Attention_49366354100559.

Multi-head attention: B=2, T=2048, D=768, H=12, Dh=64. The reference
zeroes the upper triangle of scores (not -inf) before softmax, so masked
positions contribute exp(0)=1 to both numerator and denominator.

Sharding: 8 cores = 2 batches x 4 core-groups; each core computes 3
heads of one batch and produces a partial [2048, 768] output
(pre-W_o-bias, x32 scaled); host sums the partials, rescales, adds b_o.

Precision design (rel-err budget ~7e-3 vs 2e-2 gate): per-element
relative noise propagates unchanged through random-sign contractions,
so every tensor on the V->output path is fp16 (e, vaug, suffix sums,
aout, W_o) and the V projection uses a 3-term fp8 residual expansion
(x1@W1 + x2@W1 + x1@W2) to cancel x/W quantization noise. The Q/K path
is damped by the 1/8 temperature and softmax, so x, W_q/k and q/k stay
fp8, which keeps those projections on DoubleRow matmuls (2 contraction
tiles folded per pass at 0.5 cycles/column).

Device structure:
  - x arrives host-transposed/cast to fp8 in two pair-interleaved
    layouts (QK moving operand; V stationary operand) -- no on-device
    transposes. The x quantization residual ships alongside for the V
    projection.
  - Scores^T[k, q] per 128-k-tile: plain fp8 matmuls into [128, 1024]
    psum chunks; exp on ACT straight out of PSUM (fused 1/(8*32*32)
    scale) into fp16 e-tiles; causal edge fixed by affine_select
    (fill=1) on gpsimd. Chunks are 512-aligned; garbage columns below
    the band are exp'd then overwritten by the select.
  - attn@v: plain fp16 matmuls, vaug carries a 65th all-ones column so
    the softmax denominator accumulates in the same psum rows. Fully
    masked k-quads fold in via one fp16 suffix-sum matmul (the suffix
    tile's 65th column carries the masked count).
  - finalize: fast reciprocal of row 64, partition-broadcast,
    multiply -> aout fp16 (h0/h1 stacked into one 128-row tile via a
    partition-shift DMA, h2 separate 64-row tile).
  - O-projection: fp16, two stationaries (aout01, aout2) per token
    tile; psum -> fp16 staging -> DMA out.
"""

import sys

import numpy as np

if "/opt/trn_rl_repo" not in sys.path:
    sys.path.insert(0, "/opt/trn_rl_repo")

import ml_dtypes

import concourse.mybir as mybir
from concourse import bacc
from concourse.tile import TileContext
from concourse.bass_utils import run_bass_kernel_spmd

F32 = mybir.dt.float32
F16 = mybir.dt.float16
F8 = mybir.dt.float8e4
AF = mybir.ActivationFunctionType
ALU = mybir.AluOpType
DR = mybir.MatmulPerfMode.DoubleRow
NP8 = ml_dtypes.float8_e4m3

N_CORES = 8
T = 2048
D = 768
HPC = 3   # heads per core
DH = 64
NK = 16   # k-token tiles of 128
NG = 4    # q/token groups of 512
WS = 32.0          # weight scale (W' = 32 W)
ESCALE = 0.125 / (WS * WS)


def build_nc():
    nc = bacc.Bacc("TRN2", target_bir_lowering=False, debug=False,
                   num_devices=N_CORES)
    d = {}
    ei = "ExternalInput"
    d["xq"] = nc.dram_tensor("xq", [128, 12288], F8, kind=ei).ap()
    d["xv"] = nc.dram_tensor("xv", [128, 12288], F8, kind=ei).ap()
    d["wqk"] = nc.dram_tensor("wqk", [128, 2304], F8, kind=ei).ap()
    d["wv"] = nc.dram_tensor("wv", [128, 1170], F8, kind=ei).ap()
    d["wv2"] = nc.dram_tensor("wv2", [128, 1170], F8, kind=ei).ap()
    d["woa"] = nc.dram_tensor("woa", [128, 768], F16, kind=ei).ap()
    d["wob"] = nc.dram_tensor("wob", [64, 768], F16, kind=ei).ap()
    d["bqk"] = nc.dram_tensor("bqk", [128, 3], F32, kind=ei).ap()
    d["bv"] = nc.dram_tensor("bv", [128, 195], F32, kind=ei).ap()
    d["ones"] = nc.dram_tensor("ones", [128, 512], F16, kind=ei).ap()
    d["y"] = nc.dram_tensor("y", [T, D], F16, kind="ExternalOutput").ap()
    import os
    if os.environ.get("ATTN_DEBUG"):
        eo = "ExternalOutput"
        for g in range(3):
            d[f"dbg_qkt{g}"] = nc.dram_tensor(
                f"dbg_qkt{g}", [128, T], F8, kind=eo).ap()
        for h in range(3):
            d[f"dbg_vaug{h}"] = nc.dram_tensor(
                f"dbg_vaug{h}", [128, NK * 65], F16, kind=eo).ap()
        d["dbg_aout01"] = nc.dram_tensor(
            "dbg_aout01", [128, T], F16, kind=eo).ap()
        d["dbg_aout2"] = nc.dram_tensor(
            "dbg_aout2", [64, T], F16, kind=eo).ap()
        d["dbg_e"] = nc.dram_tensor(
            "dbg_e", [128, 16 * T], F16, kind=eo).ap()
        d["dbg_pnd"] = nc.dram_tensor(
            "dbg_pnd", [128, 4 * 512], F32, kind=eo).ap()
        d["dbg_rb"] = nc.dram_tensor(
            "dbg_rb", [64, 4 * 512], F32, kind=eo).ap()

    with TileContext(nc) as tc:
        _emit(nc, tc, d)
    nc.compile()
    return nc


def _emit(nc, tc, d):
    from contextlib import ExitStack

    with ExitStack() as ctx:
        wp = ctx.enter_context(tc.tile_pool(name="wp", bufs=1))
        main = ctx.enter_context(tc.tile_pool(name="main", bufs=1))

        # ---- weights / constants ----
        wqk = wp.tile([128, 2304], F8, tag="wqk", name="wqk")
        wv = wp.tile([128, 1170], F8, tag="wv", name="wv")
        wv2 = wp.tile([128, 1170], F8, tag="wv2", name="wv2")
        woa = wp.tile([128, 768], F16, tag="woa", name="woa")
        wob = wp.tile([64, 768], F16, tag="wob", name="wob")
        bqk = wp.tile([128, 3], F32, tag="bqk", name="bqk")
        bv = wp.tile([128, 195], F32, tag="bv", name="bv")
        ones = wp.tile([128, 512], F16, tag="ones", name="ones")

        # ---- persistent SBUF ----
        qkt = [main.tile([128, T], F8, tag=f"qkt{g}", name=f"qkt{g}")
               for g in range(3)]
        alt2 = main.tile([128, T], F8, tag="alt2", name="alt2")
        # vaug[h]: [128, 16, 65] k-tile-major v values + ones column
        vaug = [main.tile([128, NK * 65], F16, tag=f"vaug{h}",
                          name=f"vaug{h}") for h in range(HPC)]
        vsum = [[main.tile([128, 65], F16, tag=f"vs{h}{g}",
                           name=f"vs{h}{g}") for g in range(3)]
                for h in range(HPC)]
        aout01 = main.tile([128, T], F16, tag="aout01", name="aout01")
        aout2 = main.tile([64, T], F16, tag="aout2", name="aout2")
        a1tmp = main.tile([64, T], F16, tag="a1tmp", name="a1tmp")

        at_ctx = ExitStack()
        ep = at_ctx.enter_context(tc.tile_pool(name="ep", bufs=20))
        fin = at_ctx.enter_context(tc.tile_pool(name="fin", bufs=2))
        yst = at_ctx.enter_context(tc.tile_pool(name="yst", bufs=3))
        sps = at_ctx.enter_context(
            tc.tile_pool(name="sps", bufs=2, space="PSUM"))
        pools = {}

        # ============ phase 0: DMAs ============
        xp_ctx = ExitStack()
        xp = xp_ctx.enter_context(tc.tile_pool(name="xp", bufs=1))
        xq = xp.tile([128, 12288], F8, tag="xq", name="xq")
        xv = xp.tile([128, 12288], F8, tag="xv", name="xv")

        s4 = lambda p: slice(4096 * p, 4096 * (p + 1))
        nc.sync.dma_start(xq[:, s4(0)], d["xq"][:, s4(0)])
        nc.gpsimd.dma_start(xq[:, s4(1)], d["xq"][:, s4(1)])
        nc.scalar.dma_start(wqk[:], d["wqk"])
        nc.scalar.dma_start(xq[:, s4(2)], d["xq"][:, s4(2)])
        nc.scalar.dma_start(bqk[:], d["bqk"])
        nc.sync.dma_start(xv[:, s4(0)], d["xv"][:, s4(0)])
        nc.gpsimd.dma_start(xv[:, s4(1)], d["xv"][:, s4(1)])
        nc.sync.dma_start(xv[:, s4(2)], d["xv"][:, s4(2)])
        nc.scalar.dma_start(wv[:], d["wv"])
        nc.scalar.dma_start(wv2[:], d["wv2"])
        nc.scalar.dma_start(bv[:], d["bv"])
        nc.scalar.dma_start(ones[:], d["ones"])
        nc.scalar.dma_start(woa[:], d["woa"])
        nc.scalar.dma_start(wob[:], d["wob"])

        # PE p-state warmup: ~3us of dummy matmuls while input DMAs land
        warm = wp.tile([128, 512], F16, tag="warm", name="warm")
        nc.vector.memset(warm[:], 1.0)
        wps = sps.tile([128, 1024], F32, tag="s", name="warmps")
        for i in range(16):
            nc.tensor.matmul(wps[:, 0:256], warm[:, 0:128], warm[:, 0:256],
                             start=True, stop=True)

        def pair(t, s, w):
            return t[:, s:s + 2 * w].rearrange("p (two c) -> p two c", two=2)

        # ============ projections ============
        pj_ctx = ExitStack()
        pj = pj_ctx.enter_context(
            tc.tile_pool(name="pj", bufs=4, space="PSUM"))

        def qk_proj(n, g):
            ps = pj.tile([128, 512], F32, tag="ps", name=f"qk{g}_{n}")
            for p in range(3):
                nc.tensor.matmul(
                    ps[:], pair(wqk, (3 * p + g) * 256, 128),
                    pair(xq, (4 * p + n) * 1024, 512),
                    start=(p == 0), stop=(p == 2), perf_mode=DR)
            nc.vector.tensor_scalar_add(
                qkt[g][:, 512 * n:512 * (n + 1)], ps[:], bqk[:, g:g + 1])

        def v_proj(tt):
            ps = pj.tile([128, 195], F32, tag="ps", name=f"v{tt}",
                         padded_shape=[128, 512])
            for i, wt in enumerate((wv, wv2)):
                for p in range(3):
                    nc.tensor.matmul(
                        ps[:], pair(xv, (16 * p + tt) * 256, 128),
                        pair(wt, 390 * p, 195),
                        start=(i == 0 and p == 0),
                        stop=(i == 1 and p == 2), perf_mode=DR)
            for h in range(HPC):
                nc.vector.tensor_add(
                    vaug[h][:, 65 * tt:65 * tt + 65],
                    ps[:, 65 * h:65 * h + 65], bv[:, 65 * h:65 * h + 65])

        def post_proj():
            nc.sync.dma_start(alt2[0:64, :], qkt[2][64:128, :])
            nc.sync.dma_start(alt2[64:128, :], qkt[2][0:64, :])
            for h in range(HPC):
                va = vaug[h].rearrange("p (k c) -> p c k", c=65)
                with nc.allow_low_precision(
                        reason="suffix sum of <=12 fp16 values"):
                    for g in range(3):
                        nc.vector.tensor_reduce(
                            vsum[h][g][:], va[:, :, 4 * (g + 1):NK],
                            axis=mybir.AxisListType.X, op=ALU.add)

        def mkcl(fn, *a):
            return lambda: fn(*a)

        # ============ attention ============
        headqk = [
            (qkt[0][0:64, :], qkt[1][0:64, :]),
            (qkt[0][64:128, :], qkt[1][64:128, :]),
            (qkt[2][0:64, :], alt2[0:64, :]),
        ]
        fill1 = nc.gpsimd.to_reg(1.0)
        etiles = {}

        def scores_quad(h, J, pending=None, pmajor=False):
            qT, kT = headqk[h]
            chunks = []
            for j in range(4):
                lo = 128 * (4 * J + j)
                for P in range(lo // 1024, 2):
                    chunks.append((j, P))
            if pmajor:
                chunks.sort(key=lambda c: (c[1], c[0]))
            share = (((len(pending) + len(chunks) - 1) // len(chunks))
                     if pending else 0)
            for (j, P) in chunks:
                ki = 4 * J + j
                lo = 128 * ki
                if (h, ki) not in etiles or etiles[(h, ki)] is None:
                    etiles[(h, ki)] = ep.tile([128, T], F16, tag="e",
                                              name=f"e{h}_{ki}")
                e = etiles[(h, ki)]
                ps = sps.tile([128, 1024], F32, tag="s",
                              name=f"s{h}_{ki}_{P}")
                for nn in range(2):
                    s0 = 1024 * P + 512 * nn
                    if s0 + 512 <= lo:
                        continue
                    a0 = max(s0, lo)
                    nc.tensor.matmul(
                        ps[:, a0 - 1024 * P:512 * (nn + 1)],
                        kT[:, lo:lo + 128], qT[:, a0:s0 + 512])
                clo = max(lo, 1024 * P)
                nc.scalar.activation(
                    e[:, clo:1024 * (P + 1)],
                    ps[:, clo - 1024 * P:1024], AF.Exp, scale=ESCALE)
                if pending:
                    for _ in range(min(share, len(pending))):
                        pending.pop(0)()
                if P == 1:
                    w = 128 * (j + 1)
                    nc.gpsimd.affine_select(
                        e[:, 512 * J:512 * J + w],
                        e[:, 512 * J:512 * J + w],
                        pattern=[[1, w]], compare_op=ALU.is_ge,
                        fill=fill1, base=512 * J - lo,
                        channel_multiplier=-1)
            while pending:
                pending.pop(0)()

        def attnv_closures(h, g):
            pnd = pools["nd"].tile([128, 512], F32, tag="nd",
                                   name=f"nd{h}{g}")
            last = 4 * g + 3
            cl = []

            def mk(ki):
                def go():
                    nc.tensor.matmul(
                        pnd[0:65, :], vaug[h][:, 65 * ki:65 * ki + 65],
                        etiles[(h, ki)][:, 512 * g:512 * (g + 1)],
                        start=(ki == 0), stop=(ki == last and g == 3))
                return go

            for ki in range(4 * g + 4):
                cl.append(mk(ki))
            if g < 3:
                cl.append(lambda: nc.tensor.matmul(
                    pnd[0:65, :], vsum[h][g][:], ones[:],
                    start=False, stop=True))
            cl.append(lambda: finalize(h, g, pnd))
            return cl

        def attnv(h, g):
            for f in attnv_closures(h, g):
                f()

        def finalize(h, g, pnd):
            dnc = fin.tile([1, 512], F32, tag="dnc", name=f"dnc{h}{g}")
            rcp = fin.tile([1, 512], F32, tag="rcp", name=f"rcp{h}{g}")
            rb = fin.tile([64, 512], F32, tag="rb", name=f"rb{h}{g}")
            nc.vector.tensor_copy(dnc[:], pnd[64:65, :])
            nc.vector.reciprocal_approx_fast(rcp[:], dnc[:])
            nc.gpsimd.partition_broadcast(rb[:], rcp[:])
            import os
            if os.environ.get("ATTN_DEBUG") and h == 2:
                dbg = fin.tile([128, 512], F32, tag="dbgp",
                               name=f"dbgp{g}", bufs=4)
                nc.vector.tensor_copy(dbg[0:65, :], pnd[0:65, :])
                nc.sync.dma_start(d["dbg_pnd"][:, 512 * g:512 * (g + 1)],
                                  dbg[:])
                nc.sync.dma_start(d["dbg_rb"][:, 512 * g:512 * (g + 1)],
                                  rb[:])
            gs = slice(512 * g, 512 * (g + 1))
            if h == 0:
                nc.vector.tensor_mul(aout01[0:64, gs], pnd[0:64, :], rb[:])
            elif h == 1:
                nc.vector.tensor_mul(a1tmp[:, gs], pnd[0:64, :], rb[:])
                nc.sync.dma_start(aout01[64:128, gs], a1tmp[:, gs])
            else:
                nc.vector.tensor_mul(aout2[:, gs], pnd[0:64, :], rb[:])

        def oproj_closures(g):
            cl = []
            for tt in range(4 * g, 4 * g + 4):
                cl.append(lambda tt=tt: oproj_tt(tt))
            return cl

        def oproj(g):
            for f in oproj_closures(g):
                f()

        def oproj_tt(tt):
            if True:
                use_act = tt >= 12
                ts = slice(128 * tt, 128 * (tt + 1))
                yt = yst.tile([128, D], F16, tag="y", name=f"y{tt}")
                po = sps.tile([128, 1024], F32, tag="s", name=f"o{tt}")
                for (c0, w) in ((0, 512), (512, 256)):
                    nc.tensor.matmul(po[:, c0:c0 + w], aout01[:, ts],
                                     woa[:, c0:c0 + w],
                                     start=True, stop=False)
                    nc.tensor.matmul(po[:, c0:c0 + w], aout2[:, ts],
                                     wob[:, c0:c0 + w],
                                     start=False, stop=True)
                if use_act:
                    nc.scalar.activation(yt[:], po[:, 0:768], AF.Copy)
                else:
                    nc.vector.tensor_copy(yt[:], po[:, 0:768])
                nc.sync.dma_start(d["y"][ts, :], yt[:])

        # ---- emission schedule ----
        qk_proj(0, 0)
        qk_proj(0, 1)
        qk_proj(1, 0)
        qk_proj(1, 1)
        pend = [mkcl(qk_proj, 2, 0), mkcl(qk_proj, 2, 1),
                mkcl(qk_proj, 3, 0), mkcl(qk_proj, 3, 1)] + [
                mkcl(qk_proj, n, 2) for n in range(NG)]
        scores_quad(0, 0, pend, pmajor=True)
        scores_quad(0, 1, [mkcl(v_proj, tt) for tt in range(0, 8)])
        scores_quad(0, 2, [mkcl(v_proj, tt) for tt in range(8, 12)])
        scores_quad(0, 3, [mkcl(v_proj, tt) for tt in range(12, 16)])
        post_proj()
        pj_ctx.close()
        xp_ctx.close()
        pools["nd"] = at_ctx.enter_context(
            tc.tile_pool(name="nd", bufs=4, space="PSUM"))

        scores_quad(1, 0, attnv_closures(0, 0) + attnv_closures(0, 1))
        scores_quad(1, 1, attnv_closures(0, 2) + attnv_closures(0, 3))
        scores_quad(1, 2, attnv_closures(1, 0))
        scores_quad(1, 3, attnv_closures(1, 1))
        scores_quad(2, 0, attnv_closures(1, 2))
        scores_quad(2, 1, attnv_closures(1, 3))
        scores_quad(2, 2, attnv_closures(2, 0) + oproj_closures(0))
        scores_quad(2, 3, attnv_closures(2, 1) + attnv_closures(2, 2)
                    + oproj_closures(1) + oproj_closures(2))
        for f in attnv_closures(2, 3) + oproj_closures(3):
            f()
        import os
        if os.environ.get("ATTN_DEBUG"):
            for ki in range(16):
                nc.sync.dma_start(d["dbg_e"][:, T * ki:T * (ki + 1)],
                                  etiles[(2, ki)][:])
            for g in range(3):
                nc.sync.dma_start(d[f"dbg_qkt{g}"], qkt[g][:])
            for h2 in range(3):
                nc.sync.dma_start(d[f"dbg_vaug{h2}"], vaug[h2][:])
            nc.sync.dma_start(d["dbg_aout01"], aout01[:])
            nc.sync.dma_start(d["dbg_aout2"], aout2[:])
        at_ctx.close()


_NC_CACHE = None


def _get_nc():
    global _NC_CACHE
    if _NC_CACHE is None:
        _NC_CACHE = build_nc()
    return _NC_CACHE


def _f8(a):
    return np.clip(np.asarray(a, dtype=np.float32),
                   -240, 240).astype(NP8)


def _make_in_maps(residual_stream, W_q, b_q, W_k, b_k, W_v, b_v, W_o, b_o):
    in_maps = []
    for c in range(N_CORES):
        b = c // 4
        hs = [3 * (c % 4) + i for i in range(HPC)]
        cs = [slice(64 * h, 64 * h + 64) for h in hs]

        xT = np.ascontiguousarray(residual_stream[b].T)  # [768, 2048]
        xT8 = _f8(xT)
        # xq: [r, p*4096 + n*1024 + i*512 + t'] = xT[256p+128i+r, 512n+t']
        xq = (xT8.reshape(3, 2, 128, 4, 512).transpose(0, 2, 3, 1, 4)
              .reshape(3, 128, 4096).transpose(1, 0, 2).reshape(128, -1))

        # xv: [r, p*4096 + tt*256 + i*128 + t'] = xT[256p+128i+r, 128tt+t']
        def vlay(a):
            return (a.reshape(3, 2, 128, 16, 128).transpose(0, 2, 3, 1, 4)
                    .reshape(3, 128, 4096).transpose(1, 0, 2)
                    .reshape(128, -1))

        xv = vlay(xT8)

        Wcat = np.concatenate(
            [W_q[:, cs[0]], W_q[:, cs[1]], W_k[:, cs[0]], W_k[:, cs[1]],
             W_q[:, cs[2]], W_k[:, cs[2]]], axis=1) * WS  # [768, 384]
        wqk = _f8(Wcat.reshape(3, 2, 128, 3, 128)
                  .transpose(2, 0, 3, 1, 4).reshape(128, 2304))

        Vcat = np.zeros((D, 195), dtype=np.float32)
        for i, s in enumerate(cs):
            Vcat[:, 65 * i:65 * i + 64] = W_v[:, s] * WS
        wv8 = _f8(Vcat).astype(np.float32)
        wvr = _f8(Vcat - wv8).astype(np.float32)
        def wvlay(a):
            return np.ascontiguousarray(
                a.reshape(3, 2, 128, 195).transpose(2, 0, 1, 3)
                .reshape(128, 1170).astype(NP8))
        wvl = wvlay(wv8)
        wv2l = wvlay(wvr)

        bqk = np.stack(
            [np.concatenate([b_q[cs[0]], b_q[cs[1]]]),
             np.concatenate([b_k[cs[0]], b_k[cs[1]]]),
             np.concatenate([b_q[cs[2]], b_k[cs[2]]])],
            axis=1).astype(np.float32) * WS
        bvv = np.zeros((195,), dtype=np.float32)
        for i, s in enumerate(cs):
            bvv[65 * i:65 * i + 64] = b_v[s] * WS
            bvv[65 * i + 64] = 1.0
        bv = np.ascontiguousarray(
            np.broadcast_to(bvv[None, :], (128, 195)).astype(np.float32))

        Wo = np.concatenate([W_o[s, :] for s in cs], axis=0)  # [192, 768]
        woa = np.ascontiguousarray(Wo[0:128, :]).astype(np.float16)
        wob = np.ascontiguousarray(Wo[128:192, :]).astype(np.float16)

        in_maps.append({
            "xq": xq, "xv": xv, "wqk": wqk,
            "wv": wvl, "wv2": wv2l, "woa": woa, "wob": wob,
            "bqk": bqk, "bv": bv,
            "ones": np.ones((128, 512), dtype=np.float16),
        })
    return in_maps


def kernel(residual_stream, W_q, b_q, W_k, b_k, W_v, b_v, W_o, b_o,
           _trace=False):
    residual_stream = np.asarray(residual_stream, dtype=np.float32)
    args = [np.asarray(a, dtype=np.float32)
            for a in (W_q, b_q, W_k, b_k, W_v, b_v, W_o, b_o)]
    W_q, b_q, W_k, b_k, W_v, b_v, W_o, b_o = args
    nc = _get_nc()
    in_maps = _make_in_maps(residual_stream, W_q, b_q, W_k, b_k, W_v, b_v,
                            W_o, b_o)
    res = run_bass_kernel_spmd(nc, in_maps, core_ids=list(range(N_CORES)),
                               trace=_trace)
    B = residual_stream.shape[0]
    out = np.zeros((B, T, D), dtype=np.float32)
    for c in range(N_CORES):
        out[c // 4] += res.results[c]["y"].astype(np.float32)
    out /= WS
    out += b_o[None, None, :]
    if _trace:
        kernel._last_result = res
    return out
